# revision 9
# baseline (speedup 1.0000x reference)
"""CompressiveMemory kernel for 8x TRN2 NeuronCores.

Math (per head h, batch b):
  phi(x) = elu(x)+1 = min(exp(x),1) + max(x,0)
  out[b,h,t,e]  = sum_d phi(Q)[b,h,t,d] M[h,d,e] / max(sum_d phi(Q)[b,h,t,d] z[h,d], eps)
  M_new[h,d,e]  = M[h,d,e] + 0.25 * sum_t (phi(K0)+phi(K1))[h,t,d] (V0+V1)[h,t,e]
  z_new[h,d]    = z[h,d]   + 0.5  * sum_t (phi(K0)+phi(K1))[h,t,d]

Sharding: heads split across the 8 cores (4 heads/core); batch kept local, so
the update (per-head state) is fully local and no collectives are needed.

Retrieve on-core dataflow per (h,b), tiles of 128 tokens:
  DMA Q chunk [128,8,128] -> ACT Exp -> DVE min(.,1) -> DVE STT (max(x,0)+ec)
  -> PE transpose (4 tiles into one PSUM bank) -> copy to SBUF
  -> PE matmul lhsT=phiQ^T[d,t] rhs=[M|z|pad][d,132] -> PSUM [t,132]
  -> DVE reciprocal of norm col -> ACT/DVE scale (division) -> SBUF -> DMA out
Update per h: phi parts on ACT/DVE/GPSIMD, sums on GPSIMD/DVE, 32 accumulating
matmuls lhsT=phiKsum[t,d], rhs=[Vsum|ones][t,129] into one PSUM bank.
"""

import numpy as np
from contextlib import ExitStack

import concourse.bass as bass
import concourse.mybir as mybir
import concourse.tile as tile
from concourse import bacc
from concourse.bass_utils import run_bass_kernel_spmd
from concourse.masks import make_identity

F32 = mybir.dt.float32
AF = mybir.ActivationFunctionType
OP = mybir.AluOpType

B = 2
H = 32
T_FULL = 4096
D = 128
N_CORES = 8
HL = H // N_CORES  # heads per core


def emit_kernel(nc, tc, ctx, hl=HL, t_len=T_FULL):
    """Emit the per-core program. Inputs/outputs are per-core head shards."""
    nt = t_len // 128          # token tiles per (b,h)
    nch = max(1, nt // 8)      # chunks of 8 tiles
    cpt = nt // nch            # tiles per chunk (8)

    q_d = nc.dram_tensor("Q", [B, hl, t_len, D], F32, kind="ExternalInput").ap()
    k_d = nc.dram_tensor("K", [B, hl, t_len, D], F32, kind="ExternalInput").ap()
    v_d = nc.dram_tensor("V", [B, hl, t_len, D], F32, kind="ExternalInput").ap()
    m_d = nc.dram_tensor("M", [hl, D, D], F32, kind="ExternalInput").ap()
    z_d = nc.dram_tensor("z", [hl, D], F32, kind="ExternalInput").ap()
    out_d = nc.dram_tensor("out", [B, hl, t_len, D], F32, kind="ExternalOutput").ap()
    mnew_d = nc.dram_tensor("M_new", [hl, D, D], F32, kind="ExternalOutput").ap()
    znew_d = nc.dram_tensor("z_new", [hl, D], F32, kind="ExternalOutput").ap()

    sb = ctx.enter_context(tc.tile_pool(name="sb", bufs=2))
    sb3 = ctx.enter_context(tc.tile_pool(name="sb3", bufs=3))
    ps = ctx.enter_context(tc.tile_pool(name="ps", bufs=2, space="PSUM"))
    ps3 = ctx.enter_context(tc.tile_pool(name="ps3", bufs=3, space="PSUM"))
    const = ctx.enter_context(tc.tile_pool(name="const", bufs=1))

    ident = const.tile([128, 128], F32, tag="ident")
    make_identity(nc, ident)

    def tview(ap4, b, h):
        # [t_len, D] -> [128, nt, D]; tile n holds tokens n*128..n*128+127
        return ap4[b, h].rearrange("(n p) d -> p n d", p=128)

    def phi_chunk(x, pool_tag, on_pool):
        """phi = min(exp(x),1) + max(x,0), elementwise on [128,cpt,128].

        on_pool=False: the max+add runs fused on DVE (scalar_tensor_tensor).
        on_pool=True: relu and add run on the otherwise-idle GPSIMD engine
        (TensorScalarPtr is not a valid Pool opcode, so no STT there).
        """
        e = sb.tile([128, cpt, D], F32, tag=pool_tag + "_e")
        nc.scalar.activation(e[:], x[:], AF.Exp)
        ec = sb.tile([128, cpt, D], F32, tag=pool_tag + "_ec")
        nc.vector.tensor_scalar(ec[:], e[:], 1.0, None, OP.min)
        ph = sb.tile([128, cpt, D], F32, tag=pool_tag + "_ph")
        if on_pool:
            s = sb.tile([128, cpt, D], F32, tag=pool_tag + "_s")
            nc.gpsimd.tensor_relu(s[:], x[:])
            nc.gpsimd.tensor_tensor(ph[:], s[:], ec[:], OP.add)
        else:
            nc.vector.scalar_tensor_tensor(ph[:], x[:], 0.0, ec[:], OP.max, OP.add)
        return ph

    for h in range(hl):
        # ---- per-head constants: [M[h] | z[h] | pad] in one SBUF tile ----
        mz = sb.tile([128, 132], F32, tag="mz")
        nc.vector.memset(mz[:, 128:132], 0.0)
        nc.sync.dma_start(mz[:, :D], m_d[h])
        with nc.allow_non_contiguous_dma(reason="tiny once-per-head z column"):
            nc.sync.dma_start(mz[:, 128:129], z_d[h, :, None])

        # ================= UPDATE (head h) =================
        psum_m = ps.tile([128, 132], F32, tag="pm")
        for c in range(nch):
            cs = slice(c * cpt, (c + 1) * cpt)
            kv = {}
            for nm, src in (("k0", k_d), ("k1", k_d), ("v0", v_d), ("v1", v_d)):
                b_idx = int(nm[1])
                tl = sb.tile([128, cpt, D], F32, tag=nm)
                nc.sync.dma_start(tl[:], tview(src, b_idx, h)[:, cs, :])
                kv[nm] = tl
            ph0 = phi_chunk(kv["k0"], "pk0", True)
            ph1 = phi_chunk(kv["k1"], "pk1", True)
            phs = sb.tile([128, cpt, D], F32, tag="phs")
            nc.gpsimd.tensor_tensor(phs[:], ph0[:], ph1[:], OP.add)
            vt = sb.tile([128, cpt, 132], F32, tag="vt")
            nc.gpsimd.memset(vt[:, :, 128:129], 1.0)
            nc.gpsimd.tensor_tensor(vt[:, :, :D], kv["v0"][:], kv["v1"][:], OP.add)
            for j in range(cpt):
                nc.tensor.matmul(
                    psum_m[:, :129],
                    lhsT=phs[:, j, :],
                    rhs=vt[:, j, :129],
                    start=(c == 0 and j == 0),
                    stop=(c == nch - 1 and j == cpt - 1),
                )
        tmp = sb.tile([128, 128], F32, tag="uptmp")
        nc.vector.tensor_scalar(tmp[:], psum_m[:, :D], 0.25, None, OP.mult)
        mnew = sb.tile([128, 128], F32, tag="mnew")
        nc.vector.tensor_tensor(mnew[:], tmp[:], mz[:, :D], OP.add)
        znew = sb.tile([128, 1], F32, tag="znew")
        nc.vector.tensor_scalar(
            znew[:], psum_m[:, 128:129], 0.5, mz[:, 128:129], OP.mult, OP.add
        )
        nc.sync.dma_start(mnew_d[h], mnew[:])
        with nc.allow_non_contiguous_dma(reason="tiny once-per-head z_new"):
            nc.sync.dma_start(znew_d[h, :, None], znew[:])

        # ================= RETRIEVE (head h, both batches) =================
        for b in range(B):
            qv = tview(q_d, b, h)
            ov = tview(out_d, b, h)
            for c in range(nch):
                cs = slice(c * cpt, (c + 1) * cpt)
                q = sb.tile([128, cpt, D], F32, tag="q")
                nc.sync.dma_start(q[:], qv[:, cs, :])
                phq = phi_chunk(q, "pq", False)
                outc = sb3.tile([128, cpt, D], F32, tag="outc")
                for tg in range(cpt // 4):  # transpose groups of 4 tiles
                    pt = ps.tile([128, 512], F32, tag="pt")
                    for s in range(4):
                        nc.tensor.transpose(
                            pt[:, s * 128:(s + 1) * 128],
                            phq[:, tg * 4 + s, :],
                            ident[:],
                        )
                    pqt = sb3.tile([128, 4, 128], F32, tag="pqt")
                    nc.any.tensor_copy(pqt[:], pt[:])
                    for mg in range(2):  # matmul groups of 2 tiles
                        pr = ps3.tile([128, 2, 132], F32, tag="pr")
                        for sl in range(2):
                            nc.tensor.matmul(
                                pr[:, sl, :],
                                lhsT=pqt[:, mg * 2 + sl, :],
                                rhs=mz[:, :132],
                                start=True,
                                stop=True,
                            )
                        rec = sb3.tile([128, 2], F32, tag="rec")
                        nc.vector.reciprocal(rec[:], pr[:, :, 128])
                        for sl in range(2):
                            jj = tg * 4 + mg * 2 + sl
                            if sl == 0:
                                nc.scalar.activation(
                                    outc[:, jj, :], pr[:, sl, :D],
                                    AF.Identity, scale=rec[:, sl:sl + 1],
                                )
                            else:
                                nc.vector.tensor_scalar(
                                    outc[:, jj, :], pr[:, sl, :D],
                                    rec[:, sl:sl + 1], None, OP.mult,
                                )
                nc.sync.dma_start(ov[:, cs, :], outc[:])


_CACHE = {}
LAST_RESULT = None  # BassKernelResults of the most recent kernel() call


def _build(hl=HL, t_len=T_FULL):
    key = (hl, t_len)
    if key not in _CACHE:
        nc = bacc.Bacc("TRN2", target_bir_lowering=False, debug=False)
        with tile.TileContext(nc) as tc:
            with ExitStack() as ctx:
                emit_kernel(nc, tc, ctx, hl=hl, t_len=t_len)
        nc.compile()
        _CACHE[key] = nc
    return _CACHE[key]


def kernel(Q, K, V, M, z):
    nc = _build()
    in_maps = []
    for i in range(N_CORES):
        hs = slice(i * HL, (i + 1) * HL)
        in_maps.append({
            "Q": np.ascontiguousarray(Q[:, hs]),
            "K": np.ascontiguousarray(K[:, hs]),
            "V": np.ascontiguousarray(V[:, hs]),
            "M": np.ascontiguousarray(M[hs]),
            "z": np.ascontiguousarray(z[hs]),
        })
    res = run_bass_kernel_spmd(nc, in_maps, core_ids=list(range(N_CORES)))
    global LAST_RESULT
    LAST_RESULT = res
    outs = res.results
    out = np.concatenate([r["out"] for r in outs], axis=1)
    m_new = np.concatenate([r["M_new"] for r in outs], axis=0)
    z_new = np.concatenate([r["z_new"] for r in outs], axis=0)
    return out, m_new, z_new


# revision 12
# speedup vs baseline: 2.1839x; 2.1839x over previous
"""CompressiveMemory kernel for 8x TRN2 NeuronCores.

Math (per head h, batch b):
  phi(x) = elu(x)+1 = min(exp(x),1) + max(x,0)
  out[b,h,t,e]  = sum_d phi(Q)[b,h,t,d] M[h,d,e] / max(sum_d phi(Q)[b,h,t,d] z[h,d], eps)
  M_new[h,d,e]  = M[h,d,e] + 0.25 * sum_t (phi(K0)+phi(K1))[h,t,d] (V0+V1)[h,t,e]
  z_new[h,d]    = z[h,d]   + 0.5  * sum_t (phi(K0)+phi(K1))[h,t,d]

Sharding: heads split across the 8 cores (4 heads/core); batch kept local, so
the per-head memory update is fully local and no collectives are needed.

Precision: inputs/outputs and all accumulations are fp32 (PSUM); the matmul
and transpose operands are bf16 (single-pass PE matmuls + fast weight load;
fp32 matmuls on TRN2 cost 2 passes + 2 weight loads each and measured ~4x
slower end-to-end).

Retrieve per (h,b), chunks of 16 token-tiles:
  DMA Q [128,16,128] -> ACT Exp(bf16) -> DVE min(.,1) -> DVE relu(Q)
  -> PE transpose-accumulate (ec^T + s^T = phi^T, 8 tiles per bf16 PSUM bank)
  -> copy to SBUF -> PE matmul lhsT=phi^T[d,t](bf16) rhs=[M|z|pad](bf16,132)
  -> fp32 PSUM [t,132] -> DVE reciprocal of norm col -> ACT/DVE scale -> DMA.
Update per h: phi parts ACT/DVE, the three elementwise sums on the otherwise
idle GPSIMD, 32 accumulating bf16 matmuls lhsT=phiKsum[t,d], rhs=[Vsum|ones].
"""

import numpy as np
from contextlib import ExitStack

import concourse.bass as bass
import concourse.mybir as mybir
import concourse.tile as tile
from concourse import bacc
from concourse.bass_utils import run_bass_kernel_spmd
from concourse.masks import make_identity

F32 = mybir.dt.float32
BF16 = mybir.dt.bfloat16
AF = mybir.ActivationFunctionType
OP = mybir.AluOpType

B = 2
H = 32
T_FULL = 4096
D = 128
N_CORES = 8
HL = H // N_CORES  # heads per core


def emit_kernel(nc, tc, ctx, hl=HL, t_len=T_FULL):
    """Emit the per-core program. Inputs/outputs are per-core head shards."""
    nt = t_len // 128           # token tiles per (b,h)
    nch = max(1, nt // 16)      # chunks of 16 tiles (1 MiB DMAs)
    cpt = nt // nch             # tiles per chunk

    q_d = nc.dram_tensor("Q", [B, hl, t_len, D], F32, kind="ExternalInput").ap()
    k_d = nc.dram_tensor("K", [B, hl, t_len, D], F32, kind="ExternalInput").ap()
    v_d = nc.dram_tensor("V", [B, hl, t_len, D], F32, kind="ExternalInput").ap()
    m_d = nc.dram_tensor("M", [hl, D, D], F32, kind="ExternalInput").ap()
    z_d = nc.dram_tensor("z", [hl, D], F32, kind="ExternalInput").ap()
    out_d = nc.dram_tensor("out", [B, hl, t_len, D], F32, kind="ExternalOutput").ap()
    mnew_d = nc.dram_tensor("M_new", [hl, D, D], F32, kind="ExternalOutput").ap()
    znew_d = nc.dram_tensor("z_new", [hl, D], F32, kind="ExternalOutput").ap()

    sb = ctx.enter_context(tc.tile_pool(name="sb", bufs=2))
    sb3 = ctx.enter_context(tc.tile_pool(name="sb3", bufs=3))
    ps = ctx.enter_context(tc.tile_pool(name="ps", bufs=2, space="PSUM"))
    ps3 = ctx.enter_context(tc.tile_pool(name="ps3", bufs=3, space="PSUM"))
    const = ctx.enter_context(tc.tile_pool(name="const", bufs=1))

    ident = const.tile([128, 128], BF16, tag="ident")
    make_identity(nc, ident)

    def tview(ap4, b, h):
        # [t_len, D] -> [128, nt, D]; tile n holds tokens n*128..n*128+127
        return ap4[b, h].rearrange("(n p) d -> p n d", p=128)

    def phi_parts(x, tag):
        """ec = min(exp(x),1), s = max(x,0), both bf16. phi = ec + s."""
        e = sb.tile([128, cpt, D], BF16, tag="phi_e")
        nc.scalar.activation(e[:], x[:], AF.Exp)
        ec = sb.tile([128, cpt, D], BF16, tag="phi_ec")
        nc.vector.tensor_scalar(ec[:], e[:], 1.0, None, OP.min)
        s = sb.tile([128, cpt, D], BF16, tag="phi_s")
        nc.vector.tensor_scalar(s[:], x[:], 0.0, None, OP.max)
        return ec, s

    for h in range(hl):
        # ---- per-head constants: [M[h] | z[h] | 0pad] fp32 + bf16 copy ----
        mz = sb.tile([128, 132], F32, tag="mz")
        nc.vector.memset(mz[:, 128:132], 0.0)
        nc.sync.dma_start(mz[:, :D], m_d[h])
        with nc.allow_non_contiguous_dma(reason="tiny once-per-head z column"):
            nc.sync.dma_start(mz[:, 128:129], z_d[h, :, None])
        mzb = sb.tile([128, 132], BF16, tag="mzb")
        nc.vector.tensor_copy(mzb[:], mz[:])

        # ================= UPDATE (head h) =================
        psum_m = ps.tile([128, 132], F32, tag="pm")
        for c in range(nch):
            cs = slice(c * cpt, (c + 1) * cpt)
            kv = {}
            for nm, src in (("k0", k_d), ("k1", k_d), ("v0", v_d), ("v1", v_d)):
                b_idx = int(nm[1])
                tl = sb.tile([128, cpt, D], F32, tag=nm)
                nc.sync.dma_start(tl[:], tview(src, b_idx, h)[:, cs, :])
                kv[nm] = tl
            ec0, s0 = phi_parts(kv["k0"], "pk0")
            ph0 = sb.tile([128, cpt, D], BF16, tag="ph0")
            nc.gpsimd.tensor_tensor(ph0[:], ec0[:], s0[:], OP.add)
            ec1, s1 = phi_parts(kv["k1"], "pk1")
            ph1 = sb.tile([128, cpt, D], BF16, tag="ph1")
            nc.gpsimd.tensor_tensor(ph1[:], ec1[:], s1[:], OP.add)
            phs = sb.tile([128, cpt, D], BF16, tag="phs")
            nc.vector.tensor_tensor(phs[:], ph0[:], ph1[:], OP.add)
            vt = sb.tile([128, cpt, 132], BF16, tag="vt")
            nc.gpsimd.memset(vt[:, :, 128:129], 1.0)
            nc.gpsimd.tensor_tensor(vt[:, :, :D], kv["v0"][:], kv["v1"][:], OP.add)
            for j in range(cpt):
                nc.tensor.matmul(
                    psum_m[:, :129],
                    lhsT=phs[:, j, :],
                    rhs=vt[:, j, :129],
                    start=(c == 0 and j == 0),
                    stop=(c == nch - 1 and j == cpt - 1),
                )
        tmp = sb.tile([128, 128], F32, tag="uptmp")
        nc.vector.tensor_scalar(tmp[:], psum_m[:, :D], 0.25, None, OP.mult)
        mnew = sb.tile([128, 128], F32, tag="mnew")
        nc.vector.tensor_tensor(mnew[:], tmp[:], mz[:, :D], OP.add)
        znew = sb.tile([128, 1], F32, tag="znew")
        nc.vector.tensor_scalar(
            znew[:], psum_m[:, 128:129], 0.5, mz[:, 128:129], OP.mult, OP.add
        )
        nc.sync.dma_start(mnew_d[h], mnew[:])
        with nc.allow_non_contiguous_dma(reason="tiny once-per-head z_new"):
            nc.sync.dma_start(znew_d[h, :, None], znew[:])

        # ================= RETRIEVE (head h, both batches) =================
        for b in range(B):
            qv = tview(q_d, b, h)
            ov = tview(out_d, b, h)
            for c in range(nch):
                cs = slice(c * cpt, (c + 1) * cpt)
                q = sb.tile([128, cpt, D], F32, tag="q")
                nc.sync.dma_start(q[:], qv[:, cs, :])
                ec, s = phi_parts(q, "pq")
                # phi in SBUF via the (retrieve-idle) GPSIMD: PSUM bf16
                # accumulation across two transposes is wrong on HW.
                phq = sb.tile([128, cpt, D], BF16, tag="phq")
                nc.gpsimd.tensor_tensor(phq[:], ec[:], s[:], OP.add)
                outc = sb3.tile([128, cpt, D], F32, tag="outc")
                for tg in range(cpt // 8):  # 8 tiles per bf16 PSUM bank
                    pt = ps.tile([128, 1024], BF16, tag="pt")
                    for t8 in range(8):
                        j = tg * 8 + t8
                        csl = slice(t8 * 128, (t8 + 1) * 128)
                        nc.tensor.matmul(
                            pt[:, csl], lhsT=phq[:, j, :], rhs=ident[:],
                            is_transpose=True, start=True, stop=True,
                        )
                    pqt = sb3.tile([128, 8, 128], BF16, tag="pqt")
                    nc.any.tensor_copy(pqt[:], pt[:])
                    for mg in range(4):  # matmul groups of 2 tiles
                        pr = ps3.tile([128, 2, 132], F32, tag="pr")
                        for sl in range(2):
                            nc.tensor.matmul(
                                pr[:, sl, :],
                                lhsT=pqt[:, mg * 2 + sl, :],
                                rhs=mzb[:, :132],
                                start=True,
                                stop=True,
                            )
                        rec = sb3.tile([128, 2], F32, tag="rec")
                        nc.vector.reciprocal(rec[:], pr[:, :, 128])
                        for sl in range(2):
                            jj = tg * 8 + mg * 2 + sl
                            if sl == 0:
                                nc.scalar.activation(
                                    outc[:, jj, :], pr[:, sl, :D],
                                    AF.Identity, scale=rec[:, sl:sl + 1],
                                )
                            else:
                                nc.vector.tensor_scalar(
                                    outc[:, jj, :], pr[:, sl, :D],
                                    rec[:, sl:sl + 1], None, OP.mult,
                                )
                nc.sync.dma_start(ov[:, cs, :], outc[:])


_CACHE = {}
LAST_RESULT = None  # BassKernelResults of the most recent kernel() call


def _build(hl=HL, t_len=T_FULL):
    key = (hl, t_len)
    if key not in _CACHE:
        nc = bacc.Bacc("TRN2", target_bir_lowering=False, debug=False)
        with tile.TileContext(nc) as tc:
            with ExitStack() as ctx:
                emit_kernel(nc, tc, ctx, hl=hl, t_len=t_len)
        nc.compile()
        _CACHE[key] = nc
    return _CACHE[key]


def kernel(Q, K, V, M, z):
    nc = _build()
    in_maps = []
    for i in range(N_CORES):
        hs = slice(i * HL, (i + 1) * HL)
        in_maps.append({
            "Q": np.ascontiguousarray(Q[:, hs]),
            "K": np.ascontiguousarray(K[:, hs]),
            "V": np.ascontiguousarray(V[:, hs]),
            "M": np.ascontiguousarray(M[hs]),
            "z": np.ascontiguousarray(z[hs]),
        })
    res = run_bass_kernel_spmd(nc, in_maps, core_ids=list(range(N_CORES)))
    global LAST_RESULT
    LAST_RESULT = res
    outs = res.results
    out = np.concatenate([r["out"] for r in outs], axis=1)
    m_new = np.concatenate([r["M_new"] for r in outs], axis=0)
    z_new = np.concatenate([r["z_new"] for r in outs], axis=0)
    return out, m_new, z_new
